# revision 3
# baseline (speedup 1.0000x reference)
# Trainium2 Bass kernel for nn_BiLSTM_encoder_decoder (batch-sharded over 8 cores).
#
# Strategy (per core, batch shard B=32):
#   - All matmuls in bf16 with fp32 PSUM accumulation; cell state c kept fp32.
#   - "Quartered" gate layout: gates PSUM tile G[128, 512] where partition
#     rows = (quarter q, batch b) and free = (gate slot, within-quarter col).
#     Each of the 4 PE column-groups (tile_position=(0,32q)) computes one
#     quarter, so M=32 batch matmuls stream weights at full PE rate.
#   - h is kept transposed (hT[128, 128] = h.T arranged [hdim-in-quarter,
#     (quarter, batch)]) so it is directly the stationary operand of the next
#     step's matmul.  One full-mode 128x128 PE transpose per cell step
#     produces it.
#   - x inputs and biases folded in as an extra K=4 matmul round with a
#     constant-1 row.
#   - Decoder output linear + "decoder_sub" feedback folded into a single
#     5-round matmul producing the next decoder input [o0; s1; s2; 1]^T.
import os
import sys

for _p in ("/opt/trn_rl_repo", "/root/.axon_site/_ro/trn_rl_repo"):
    if os.path.isdir(_p) and _p not in sys.path:
        sys.path.append(_p)

import numpy as np
import ml_dtypes

BF16 = np.float16  # fp16: same PE rate/bytes as bf16, 8x finer mantissa

# ---- problem constants (hardcoded from the spec) ----
H = 512          # hidden
Q = 4            # quarters
B = 32           # batch per core
N_CORES = 8
FULL_B = 256
T = int(os.environ.get("BASS_LSTM_T", "128"))     # encoder seq len
TL = int(os.environ.get("BASS_LSTM_TL", "32"))    # decoder len
IN_SZ = 3

# gate slots in the free dim of G: slot0=i, slot1=f, slot2=o, slot3=g
ORIG_OF_SLOT = [0, 1, 3, 2]

_NC_CACHE = {}


# ---------------------------------------------------------------- host prep
def _prep_rounds(W):
    """W [2048, Kin] (pytorch gate order i,f,g,o) -> [R, 4q, 128, 512]."""
    Kin = W.shape[1]
    R = Kin // 128
    out = np.empty((R, 4, 128, 512), np.float32)
    for slot in range(4):
        orig = ORIG_OF_SLOT[slot]
        for q in range(4):
            blk = W[512 * orig + 128 * q : 512 * orig + 128 * (q + 1), :]  # [128cc, Kin]
            out[:, q, :, 128 * slot : 128 * (slot + 1)] = (
                blk.T.reshape(Kin // 128, 128, 128)
            )
    return out


def _prep_x(Wih, bias):
    """Wih [2048, n_in<=3] or None, bias [2048] -> [4 rows, 4q, 512].

    Row j<3 = Wih.T row j; row 3 = bias (multiplied by the constant-1 row)."""
    out = np.zeros((4, 4, 512), np.float32)
    for slot in range(4):
        orig = ORIG_OF_SLOT[slot]
        for q in range(4):
            sl = slice(512 * orig + 128 * q, 512 * orig + 128 * (q + 1))
            if Wih is not None:
                n_in = Wih.shape[1]
                out[0:n_in, q, 128 * slot : 128 * (slot + 1)] = Wih[sl, :].T
            out[3, q, 128 * slot : 128 * (slot + 1)] = bias[sl]
    return out


def _host_prep_weights(enc_params, dec_params, lin_W, lin_b):
    ws = {}
    # L1: 2 dirs. Whh rounds + x rounds.
    wl1 = np.empty((128, 2, 4, 4, 512), np.float32)
    wx_l1 = np.empty((4, 2, 4, 512), np.float32)
    for u in range(2):
        Wih, Whh, bih, bhh = [np.asarray(a, np.float32) for a in enc_params[0][u]]
        r = _prep_rounds(Whh)                      # [4, 4, 128, 512]
        wl1[:, u] = r.transpose(2, 0, 1, 3)        # [128, r, q, 512]
        wx_l1[:, u] = _prep_x(Wih, bih + bhh)
    # L2: rounds 0-3 Whh, 4-11 Wih(K=1024), bias-only x round.
    wl2r = np.empty((128, 2, 4, 4, 512), np.float32)
    wl2i = np.empty((128, 2, 8, 4, 512), np.float32)
    wx_l2b = np.empty((4, 2, 4, 512), np.float32)
    for u in range(2):
        Wih, Whh, bih, bhh = [np.asarray(a, np.float32) for a in enc_params[1][u]]
        wl2r[:, u] = _prep_rounds(Whh).transpose(2, 0, 1, 3)
        wl2i[:, u] = _prep_rounds(Wih).transpose(2, 0, 1, 3)
        wx_l2b[:, u] = _prep_x(None, bih + bhh)
    # decoder: 4 layers
    wdr = np.empty((128, 4, 4, 4, 512), np.float32)
    wdi = np.empty((128, 3, 4, 4, 512), np.float32)
    wx_db = np.empty((4, 3, 4, 512), np.float32)
    for l in range(4):
        Wih, Whh, bih, bhh = [np.asarray(a, np.float32) for a in dec_params[l]]
        wdr[:, l] = _prep_rounds(Whh).transpose(2, 0, 1, 3)
        if l == 0:
            wx_d0 = _prep_x(Wih, bih + bhh)        # [4, 4, 512]
        else:
            wdi[:, l - 1] = _prep_rounds(Wih).transpose(2, 0, 1, 3)
            wx_db[:, l - 1] = _prep_x(None, bih + bhh)
    # feedback W*: x_new^T [4, 32] = sum_q w_fb[:, q].T @ hT[:, q] + wx_fb.T @ x_prevT
    lw = np.asarray(lin_W, np.float32)[0]          # [512]
    lb = float(np.asarray(lin_b, np.float32)[0])
    w_fb = np.zeros((128, 16), np.float32)         # [kk, 4q + m]
    for q in range(4):
        seg = lw[128 * q : 128 * (q + 1)]
        w_fb[:, 4 * q + 0] = seg
        w_fb[:, 4 * q + 1] = -seg
        w_fb[:, 4 * q + 2] = seg
    wx_fb = np.zeros((4, 4), np.float32)
    wx_fb[0, 1] = 1.0
    wx_fb[0, 2] = -1.0
    wx_fb[1, 2] = 1.0
    wx_fb[3, 0] = lb
    wx_fb[3, 1] = -lb
    wx_fb[3, 2] = lb
    wx_fb[3, 3] = 1.0
    # identity for PE transpose + w_fb packed in one always-live tile
    idw = np.zeros((128, 144), np.float32)
    idw[:, 0:128] = np.eye(128, dtype=np.float32)
    idw[:, 128:144] = w_fb

    ws["wl1"] = wl1.astype(BF16)
    ws["wl2r"] = wl2r.astype(BF16)
    ws["wl2i"] = wl2i.astype(BF16)
    ws["wdr"] = wdr.astype(BF16)
    ws["wdi"] = wdi.astype(BF16)
    ws["idw"] = idw.astype(BF16)
    # smallsB [4, 4128]: ones432 [0:32] | wx_l2b [32:4128] (u*2048 + q*512)
    sb_ = np.zeros((4, 4128), np.float32)
    sb_[3, 0:32] = 1.0
    sb_[:, 32:4128] = wx_l2b.reshape(4, 4096)
    ws["smallsB"] = sb_.astype(BF16)
    # smallsC [4, 8256]: wx_d0 [0:2048] | wx_db [2048:8192] | wx_fb [8192:8196]
    # (x0t [8224:8256] is per-core, filled later)
    sc = np.zeros((4, 8256), np.float32)
    sc[:, 0:2048] = wx_d0.reshape(4, 2048)
    sc[:, 2048:8192] = wx_db.reshape(4, 6144)
    sc[:, 8192:8196] = wx_fb
    ws["smallsC_base"] = sc
    # smallsA [4, 4096 + 2048]: xt4 [0:32T] per-core | wx_l1 [32T : 32T+4096]
    ws["wx_l1_flat"] = wx_l1.reshape(4, 4096)
    return ws


# ---------------------------------------------------------------- device build
def _build_nc():
    import concourse.bass as bass
    import concourse.bacc as bacc
    import concourse.tile as tile
    from concourse import mybir

    f32 = mybir.dt.float32
    bf16 = mybir.dt.float16
    SIG = mybir.ActivationFunctionType.Sigmoid
    TANH = mybir.ActivationFunctionType.Tanh

    nc = bacc.Bacc("TRN2", target_bir_lowering=False, debug=False)

    d = {}
    d["wl1"] = nc.dram_tensor("wl1", [128, 2, 4, 4, 512], bf16, kind="ExternalInput")
    d["wl2r"] = nc.dram_tensor("wl2r", [128, 2, 4, 4, 512], bf16, kind="ExternalInput")
    d["wl2i"] = nc.dram_tensor("wl2i", [128, 2, 8, 4, 512], bf16, kind="ExternalInput")
    d["wdr"] = nc.dram_tensor("wdr", [128, 4, 4, 4, 512], bf16, kind="ExternalInput")
    d["wdi"] = nc.dram_tensor("wdi", [128, 3, 4, 4, 512], bf16, kind="ExternalInput")
    d["idw"] = nc.dram_tensor("idw", [128, 144], bf16, kind="ExternalInput")
    d["smallsA"] = nc.dram_tensor("smallsA", [4, 32 * T + 4096], bf16, kind="ExternalInput")
    d["smallsB"] = nc.dram_tensor("smallsB", [4, 4128], bf16, kind="ExternalInput")
    d["smallsC"] = nc.dram_tensor("smallsC", [4, 8256], bf16, kind="ExternalInput")
    d_out = nc.dram_tensor("out", [1, 32 * TL], f32, kind="ExternalOutput")

    XT_OFF = 0            # smallsA: xt4 [0 : 32T], wx_l1 [32T : 32T+4096]
    WXL1_OFF = 32 * T

    with tile.TileContext(nc) as tc:
        import contextlib

        with contextlib.ExitStack() as ctx:
            big = ctx.enter_context(tc.tile_pool(name="big", bufs=1))
            state = ctx.enter_context(tc.tile_pool(name="state", bufs=1))
            work = ctx.enter_context(tc.tile_pool(name="work", bufs=3))
            htp = ctx.enter_context(tc.tile_pool(name="htp", bufs=4))
            xpp = ctx.enter_context(tc.tile_pool(name="xpp", bufs=2))
            gp = ctx.enter_context(tc.tile_pool(name="gp", bufs=4, space="PSUM"))
            tp = ctx.enter_context(tc.tile_pool(name="tp", bufs=2, space="PSUM"))
            fbp = ctx.enter_context(tc.tile_pool(name="fbp", bufs=2, space="PSUM"))

            # ---- persistent tiles
            idw = big.tile([128, 144], bf16, tag="idw")
            smallsB = big.tile([4, 4128], bf16, tag="smB")
            smallsA = big.tile([4, 32 * T + 4096], bf16, tag="sA")
            wl1 = big.tile([128, 2, 4, 4, 512], bf16, tag="wB")
            hT1 = big.tile([128, 2, T, 128], bf16, tag="wD")
            nc.sync.dma_start(idw[:], d["idw"][:])
            nc.sync.dma_start(smallsB[:], d["smallsB"][:])
            nc.sync.dma_start(smallsA[:], d["smallsA"][:])
            for u in range(2):
                for r in range(4):
                    nc.sync.dma_start(wl1[:, u, r], d["wl1"][:, u, r])

            ident = idw[:, 0:128]
            ones4 = smallsB[0:4, 0:32]

            c_tiles = [state.tile([128, 128], f32, tag=f"c{u}", name=f"c{u}") for u in range(4)]

            def cell_ew(G, cu, s0, Hn_tag="Hn"):
                """Elementwise LSTM cell on G psum tile; returns Hn (bf16)."""
                S = work.tile([128, 384], bf16, tag="S")
                TG = work.tile([128, 128], bf16, tag="TG")
                nc.scalar.activation(S[:], G[:, 0:384], SIG)
                nc.scalar.activation(TG[:], G[:, 384:512], TANH)
                P1 = work.tile([128, 128], f32, tag="P1")
                nc.vector.tensor_mul(P1[:], S[:, 0:128], TG[:])
                if s0:
                    nc.vector.tensor_copy(cu[:], P1[:])
                else:
                    P2 = work.tile([128, 128], f32, tag="P2")
                    nc.vector.tensor_mul(P2[:], S[:, 128:256], cu[:])
                    nc.vector.tensor_add(cu[:], P1[:], P2[:])
                TC = work.tile([128, 128], bf16, tag="TC")
                nc.scalar.activation(TC[:], cu[:], TANH)
                Hn = work.tile([128, 128], bf16, tag=Hn_tag)
                nc.vector.tensor_mul(Hn[:], S[:, 256:384], TC[:])
                return Hn

            def transpose_h(Hn):
                HT = tp.tile([128, 128], bf16, tag="HT")
                nc.tensor.transpose(HT[:], Hn[:], ident)
                return HT

            MM = nc.tensor.matmul

            # ================= encoder layer 1 =================
            for s in range(T):
                for u in range(2):
                    t = s if u == 0 else T - 1 - s
                    t_prev = s - 1 if u == 0 else T - s
                    G = gp.tile([128, 512], f32, tag="G")
                    if s > 0:
                        for r in range(4):
                            for q in range(4):
                                MM(G[32 * q : 32 * q + 32, :],
                                   hT1[:, u, t_prev, 32 * r : 32 * r + 32],
                                   wl1[:, u, r, q, :],
                                   start=(r == 0), stop=False,
                                   tile_position=(0, 32 * q), skip_group_check=True)
                    xt = smallsA[0:4, XT_OFF + 32 * t : XT_OFF + 32 * t + 32]
                    for q in range(4):
                        MM(G[32 * q : 32 * q + 32, :],
                           xt,
                           smallsA[0:4, WXL1_OFF + u * 2048 + 512 * q : WXL1_OFF + u * 2048 + 512 * (q + 1)],
                           start=(s == 0), stop=True,
                           tile_position=(0, 32 * q), skip_group_check=True)
                    Hn = cell_ew(G, c_tiles[u], s == 0)
                    HT = transpose_h(Hn)
                    nc.vector.tensor_copy(hT1[:, u, t, :], HT[:])

            # ---- L2 weights (reuse slots of wl1 / new slot) ----
            wl2r = big.tile([128, 2, 4, 4, 512], bf16, tag="wB")
            wl2i = big.tile([128, 2, 8, 4, 512], bf16, tag="wC")
            for u in range(2):
                for r in range(4):
                    nc.sync.dma_start(wl2r[:, u, r], d["wl2r"][:, u, r])
                for r in range(8):
                    nc.sync.dma_start(wl2i[:, u, r], d["wl2i"][:, u, r])

            # ================= encoder layer 2 =================
            hT2_last = [None, None]
            for s in range(T):
                for u in range(2):
                    t = s if u == 0 else T - 1 - s
                    G = gp.tile([128, 512], f32, tag="G")
                    if s > 0:
                        hprev = hT2_last[u]
                        for r in range(4):
                            for q in range(4):
                                MM(G[32 * q : 32 * q + 32, :],
                                   hprev[:, 32 * r : 32 * r + 32],
                                   wl2r[:, u, r, q, :],
                                   start=(r == 0), stop=False,
                                   tile_position=(0, 32 * q), skip_group_check=True)
                    for k in range(8):
                        src_u, rr = (0, k) if k < 4 else (1, k - 4)
                        for q in range(4):
                            MM(G[32 * q : 32 * q + 32, :],
                               hT1[:, src_u, t, 32 * rr : 32 * rr + 32],
                               wl2i[:, u, k, q, :],
                               start=(s == 0 and k == 0), stop=False,
                               tile_position=(0, 32 * q), skip_group_check=True)
                    for q in range(4):
                        MM(G[32 * q : 32 * q + 32, :],
                           ones4,
                           smallsB[0:4, 32 + u * 2048 + 512 * q : 32 + u * 2048 + 512 * (q + 1)],
                           start=False, stop=True,
                           tile_position=(0, 32 * q), skip_group_check=True)
                    Hn = cell_ew(G, c_tiles[2 + u], s == 0)
                    HT = transpose_h(Hn)
                    h2 = htp.tile([128, 128], bf16, tag=f"h2_{u}")
                    nc.vector.tensor_copy(h2[:], HT[:])
                    hT2_last[u] = h2

            # ---- decoder init state (copy before slot reuse) ----
            hTd = [state.tile([128, 128], bf16, tag=f"hd{l}", name=f"hd{l}") for l in range(4)]
            nc.vector.tensor_copy(hTd[0][:], hT1[:, 0, T - 1, :])
            nc.vector.tensor_copy(hTd[1][:], hT1[:, 1, 0, :])
            nc.vector.tensor_copy(hTd[2][:], hT2_last[0][:])
            nc.vector.tensor_copy(hTd[3][:], hT2_last[1][:])
            c_dec = c_tiles  # layer l <- unit order [L1f, L1b, L2f, L2b]

            # ---- decoder weights (reuse slots) ----
            wdr = big.tile([128, 4, 4, 4, 512], bf16, tag="wD")
            wdi = big.tile([128, 3, 4, 4, 512], bf16, tag="wC")
            smallsC = big.tile([4, 8256], bf16, tag="sA")
            for l in range(4):
                for r in range(4):
                    nc.sync.dma_start(wdr[:, l, r], d["wdr"][:, l, r])
            for l in range(3):
                for r in range(4):
                    nc.sync.dma_start(wdi[:, l, r], d["wdi"][:, l, r])
            nc.sync.dma_start(smallsC[:], d["smallsC"][:])

            outbuf = state.tile([1, 32 * TL], f32, tag="outb")
            xp = xpp.tile([4, 32], bf16, tag="xp")
            nc.vector.tensor_copy(xp[:], smallsC[0:4, 8224:8256])

            # ================= decoder =================
            for td in range(TL):
                Gd = []
                # pre-rounds: Whh (+bias / +x) for all layers
                for l in range(4):
                    G = gp.tile([128, 512], f32, tag="G")
                    Gd.append(G)
                    for r in range(4):
                        for q in range(4):
                            MM(G[32 * q : 32 * q + 32, :],
                               hTd[l][:, 32 * r : 32 * r + 32],
                               wdr[:, l, r, q, :],
                               start=(r == 0), stop=False,
                               tile_position=(0, 32 * q), skip_group_check=True)
                    if l == 0:
                        for q in range(4):
                            MM(G[32 * q : 32 * q + 32, :],
                               xp,
                               smallsC[0:4, 512 * q : 512 * (q + 1)],
                               start=False, stop=True,
                               tile_position=(0, 32 * q), skip_group_check=True)
                    else:
                        for q in range(4):
                            MM(G[32 * q : 32 * q + 32, :],
                               ones4,
                               smallsC[0:4, 2048 + (l - 1) * 2048 + 512 * q : 2048 + (l - 1) * 2048 + 512 * (q + 1)],
                               start=False, stop=False,
                               tile_position=(0, 32 * q), skip_group_check=True)
                # chain
                for l in range(4):
                    if l > 0:
                        for r in range(4):
                            for q in range(4):
                                MM(Gd[l][32 * q : 32 * q + 32, :],
                                   hTd[l - 1][:, 32 * r : 32 * r + 32],
                                   wdi[:, l - 1, r, q, :],
                                   start=False, stop=(r == 3),
                                   tile_position=(0, 32 * q), skip_group_check=True)
                    Hn = cell_ew(Gd[l], c_dec[l], False)
                    HT = transpose_h(Hn)
                    hd_new = htp.tile([128, 128], bf16, tag=f"hd_cur{l}")
                    nc.vector.tensor_copy(hd_new[:], HT[:])
                    hTd[l] = hd_new
                # feedback: x_new^T [4, 32] = W* @ [h3; x_prev]
                Pfb = fbp.tile([4, 32], f32, tag="fb")
                for q in range(4):
                    MM(Pfb[:],
                       idw[:, 128 + 4 * q : 128 + 4 * (q + 1)],
                       hTd[3][:, 32 * q : 32 * q + 32],
                       start=(q == 0), stop=False,
                       tile_position=(0, 0), skip_group_check=True)
                MM(Pfb[:],
                   smallsC[0:4, 8192:8196],
                   xp,
                   start=False, stop=True,
                   tile_position=(0, 0), skip_group_check=True)
                xp = xpp.tile([4, 32], bf16, tag="xp")
                nc.vector.tensor_copy(xp[:], Pfb[:])
                nc.vector.tensor_copy(outbuf[0:1, 32 * td : 32 * td + 32], Pfb[0:1, :])

            nc.sync.dma_start(d_out[:], outbuf[:])

    nc.compile()
    return nc


# ---------------------------------------------------------------- entry
def _run(inputs, trace=False, tmpdir=None):
    enc_params = inputs["enc_params"]
    dec_params = inputs["dec_params"]
    lin_W = np.asarray(inputs["lin_W"], np.float32)
    lin_b = np.asarray(inputs["lin_b"], np.float32)
    x = np.asarray(inputs["x"], np.float32)
    tl = int(np.asarray(inputs["target_len"]))
    assert tl == TL, f"target_len {tl} != {TL}"
    assert x.shape == (FULL_B, T, IN_SZ), x.shape

    if "nc" not in _NC_CACHE:
        _NC_CACHE["nc"] = _build_nc()
    nc = _NC_CACHE["nc"]

    ws = _host_prep_weights(enc_params, dec_params, lin_W, lin_b)

    in_maps = []
    for s in range(N_CORES):
        bsl = slice(s * B, (s + 1) * B)
        xs = x[bsl]  # [32, T, 3]
        sa = np.zeros((4, 32 * T + 4096), np.float32)
        # xt4: [j, 32t+b]
        sa[0:3, 0 : 32 * T] = xs.transpose(2, 1, 0).reshape(3, T * 32)
        sa[3, 0 : 32 * T] = 1.0
        sa[:, 32 * T :] = ws["wx_l1_flat"]
        sc = ws["smallsC_base"].copy()
        sc[0:3, 8224:8256] = xs[:, T - 1, :].T
        sc[3, 8224:8256] = 1.0
        in_maps.append({
            "wl1": ws["wl1"], "wl2r": ws["wl2r"], "wl2i": ws["wl2i"],
            "wdr": ws["wdr"], "wdi": ws["wdi"], "idw": ws["idw"],
            "smallsA": sa.astype(BF16), "smallsB": ws["smallsB"],
            "smallsC": sc.astype(BF16),
        })

    from concourse.bass_utils import run_bass_kernel_spmd

    if trace:
        _install_ntff_hook()
    res = run_bass_kernel_spmd(
        nc, in_maps, core_ids=list(range(N_CORES)), trace=trace, tmpdir=tmpdir
    )
    out = np.empty((FULL_B, TL, 1), np.float32)
    for s in range(N_CORES):
        o = np.asarray(res.results[s]["out"]).reshape(TL, B)  # [t, b]
        out[s * B : (s + 1) * B, :, 0] = o.T
    return out, res


def _install_ntff_hook():
    import types
    import antenv

    if "antenv.axon_hooks" in sys.modules:
        return
    hooks_mod = types.ModuleType("antenv.axon_hooks")
    _HOOK = [None]
    hooks_mod.set_axon_ntff_profile_hook = lambda h: _HOOK.__setitem__(0, h)
    hooks_mod.get_axon_ntff_profile_hook = lambda: _HOOK[0]
    sys.modules["antenv.axon_hooks"] = hooks_mod
    antenv.axon_hooks = hooks_mod
    try:
        from trn_agent_boot.trn_boot import _ntff_profile_via_ctypes

        hooks_mod.set_axon_ntff_profile_hook(
            _ntff_profile_via_ctypes("/opt/axon/libaxon_pjrt.so")
        )
    except Exception:
        pass


def kernel(**inputs):
    out, _ = _run(inputs, trace=False)
    return out


# revision 4
# speedup vs baseline: 1.0108x; 1.0108x over previous
# Trainium2 Bass kernel for nn_BiLSTM_encoder_decoder (batch-sharded over 8 cores).
#
# Strategy (per core, batch shard B=32):
#   - All matmuls in bf16 with fp32 PSUM accumulation; cell state c kept fp32.
#   - "Quartered" gate layout: gates PSUM tile G[128, 512] where partition
#     rows = (quarter q, batch b) and free = (gate slot, within-quarter col).
#     Each of the 4 PE column-groups (tile_position=(0,32q)) computes one
#     quarter, so M=32 batch matmuls stream weights at full PE rate.
#   - h is kept transposed (hT[128, 128] = h.T arranged [hdim-in-quarter,
#     (quarter, batch)]) so it is directly the stationary operand of the next
#     step's matmul.  One full-mode 128x128 PE transpose per cell step
#     produces it.
#   - x inputs and biases folded in as an extra K=4 matmul round with a
#     constant-1 row.
#   - Decoder output linear + "decoder_sub" feedback folded into a single
#     5-round matmul producing the next decoder input [o0; s1; s2; 1]^T.
import os
import sys

for _p in ("/opt/trn_rl_repo", "/root/.axon_site/_ro/trn_rl_repo"):
    if os.path.isdir(_p) and _p not in sys.path:
        sys.path.append(_p)

import numpy as np
import ml_dtypes

BF16 = np.float16  # fp16: same PE rate/bytes as bf16, 8x finer mantissa

# ---- problem constants (hardcoded from the spec) ----
H = 512          # hidden
Q = 4            # quarters
B = 32           # batch per core
N_CORES = 8
FULL_B = 256
T = int(os.environ.get("BASS_LSTM_T", "128"))     # encoder seq len
TL = int(os.environ.get("BASS_LSTM_TL", "32"))    # decoder len
IN_SZ = 3

# gate slots in the free dim of G: slot0=i, slot1=f, slot2=o, slot3=g
ORIG_OF_SLOT = [0, 1, 3, 2]

_NC_CACHE = {}


# ---------------------------------------------------------------- host prep
def _prep_rounds(W):
    """W [2048, Kin] (pytorch gate order i,f,g,o) -> [R, 4q, 128, 512]."""
    Kin = W.shape[1]
    R = Kin // 128
    out = np.empty((R, 4, 128, 512), np.float32)
    for slot in range(4):
        orig = ORIG_OF_SLOT[slot]
        for q in range(4):
            blk = W[512 * orig + 128 * q : 512 * orig + 128 * (q + 1), :]  # [128cc, Kin]
            out[:, q, :, 128 * slot : 128 * (slot + 1)] = (
                blk.T.reshape(Kin // 128, 128, 128)
            )
    return out


def _prep_x(Wih, bias):
    """Wih [2048, n_in<=3] or None, bias [2048] -> [4 rows, 4q, 512].

    Row j<3 = Wih.T row j; row 3 = bias (multiplied by the constant-1 row)."""
    out = np.zeros((4, 4, 512), np.float32)
    for slot in range(4):
        orig = ORIG_OF_SLOT[slot]
        for q in range(4):
            sl = slice(512 * orig + 128 * q, 512 * orig + 128 * (q + 1))
            if Wih is not None:
                n_in = Wih.shape[1]
                out[0:n_in, q, 128 * slot : 128 * (slot + 1)] = Wih[sl, :].T
            out[3, q, 128 * slot : 128 * (slot + 1)] = bias[sl]
    return out


def _host_prep_weights(enc_params, dec_params, lin_W, lin_b):
    ws = {}
    # L1: 2 dirs. Whh rounds + x rounds.
    wl1 = np.empty((128, 2, 4, 4, 512), np.float32)
    wx_l1 = np.empty((4, 2, 4, 512), np.float32)
    for u in range(2):
        Wih, Whh, bih, bhh = [np.asarray(a, np.float32) for a in enc_params[0][u]]
        r = _prep_rounds(Whh)                      # [4, 4, 128, 512]
        wl1[:, u] = r.transpose(2, 0, 1, 3)        # [128, r, q, 512]
        wx_l1[:, u] = _prep_x(Wih, bih + bhh)
    # L2: rounds 0-3 Whh, 4-11 Wih(K=1024), bias-only x round.
    wl2r = np.empty((128, 2, 4, 4, 512), np.float32)
    wl2i = np.empty((128, 2, 8, 4, 512), np.float32)
    wx_l2b = np.empty((4, 2, 4, 512), np.float32)
    for u in range(2):
        Wih, Whh, bih, bhh = [np.asarray(a, np.float32) for a in enc_params[1][u]]
        wl2r[:, u] = _prep_rounds(Whh).transpose(2, 0, 1, 3)
        wl2i[:, u] = _prep_rounds(Wih).transpose(2, 0, 1, 3)
        wx_l2b[:, u] = _prep_x(None, bih + bhh)
    # decoder: 4 layers
    wdr = np.empty((128, 4, 4, 4, 512), np.float32)
    wdi = np.empty((128, 3, 4, 4, 512), np.float32)
    wx_db = np.empty((4, 3, 4, 512), np.float32)
    for l in range(4):
        Wih, Whh, bih, bhh = [np.asarray(a, np.float32) for a in dec_params[l]]
        wdr[:, l] = _prep_rounds(Whh).transpose(2, 0, 1, 3)
        if l == 0:
            wx_d0 = _prep_x(Wih, bih + bhh)        # [4, 4, 512]
        else:
            wdi[:, l - 1] = _prep_rounds(Wih).transpose(2, 0, 1, 3)
            wx_db[:, l - 1] = _prep_x(None, bih + bhh)
    # feedback W*: x_new^T [4, 32] = sum_q w_fb[:, q].T @ hT[:, q] + wx_fb.T @ x_prevT
    lw = np.asarray(lin_W, np.float32)[0]          # [512]
    lb = float(np.asarray(lin_b, np.float32)[0])
    w_fb = np.zeros((128, 16), np.float32)         # [kk, 4q + m]
    for q in range(4):
        seg = lw[128 * q : 128 * (q + 1)]
        w_fb[:, 4 * q + 0] = seg
        w_fb[:, 4 * q + 1] = -seg
        w_fb[:, 4 * q + 2] = seg
    wx_fb = np.zeros((4, 4), np.float32)
    wx_fb[0, 1] = 1.0
    wx_fb[0, 2] = -1.0
    wx_fb[1, 2] = 1.0
    wx_fb[3, 0] = lb
    wx_fb[3, 1] = -lb
    wx_fb[3, 2] = lb
    wx_fb[3, 3] = 1.0
    # identity for PE transpose + w_fb packed in one always-live tile
    idw = np.zeros((128, 144), np.float32)
    idw[:, 0:128] = np.eye(128, dtype=np.float32)
    idw[:, 128:144] = w_fb

    ws["wl1"] = wl1.astype(BF16)
    ws["wl2r"] = wl2r.astype(BF16)
    ws["wl2i"] = wl2i.astype(BF16)
    ws["wdr"] = wdr.astype(BF16)
    ws["wdi"] = wdi.astype(BF16)
    ws["idw"] = idw.astype(BF16)
    # smallsB [4, 4128]: ones432 [0:32] | wx_l2b [32:4128] (u*2048 + q*512)
    sb_ = np.zeros((4, 4128), np.float32)
    sb_[3, 0:32] = 1.0
    sb_[:, 32:4128] = wx_l2b.reshape(4, 4096)
    ws["smallsB"] = sb_.astype(BF16)
    # smallsC [4, 8256]: wx_d0 [0:2048] | wx_db [2048:8192] | wx_fb [8192:8196]
    # (x0t [8224:8256] is per-core, filled later)
    sc = np.zeros((4, 8256), np.float32)
    sc[:, 0:2048] = wx_d0.reshape(4, 2048)
    sc[:, 2048:8192] = wx_db.reshape(4, 6144)
    sc[:, 8192:8196] = wx_fb
    ws["smallsC_base"] = sc
    # smallsA [4, 4096 + 2048]: xt4 [0:32T] per-core | wx_l1 [32T : 32T+4096]
    ws["wx_l1_flat"] = wx_l1.reshape(4, 4096)
    return ws


# ---------------------------------------------------------------- device build
def _build_nc():
    import concourse.bass as bass
    import concourse.bacc as bacc
    import concourse.tile as tile
    from concourse import mybir

    f32 = mybir.dt.float32
    bf16 = mybir.dt.float16
    SIG = mybir.ActivationFunctionType.Sigmoid
    TANH = mybir.ActivationFunctionType.Tanh

    nc = bacc.Bacc("TRN2", target_bir_lowering=False, debug=False)

    d = {}
    d["wl1"] = nc.dram_tensor("wl1", [128, 2, 4, 4, 512], bf16, kind="ExternalInput")
    d["wl2r"] = nc.dram_tensor("wl2r", [128, 2, 4, 4, 512], bf16, kind="ExternalInput")
    d["wl2i"] = nc.dram_tensor("wl2i", [128, 2, 8, 4, 512], bf16, kind="ExternalInput")
    d["wdr"] = nc.dram_tensor("wdr", [128, 4, 4, 4, 512], bf16, kind="ExternalInput")
    d["wdi"] = nc.dram_tensor("wdi", [128, 3, 4, 4, 512], bf16, kind="ExternalInput")
    d["idw"] = nc.dram_tensor("idw", [128, 144], bf16, kind="ExternalInput")
    d["smallsA"] = nc.dram_tensor("smallsA", [4, 32 * T + 4096], bf16, kind="ExternalInput")
    d["smallsB"] = nc.dram_tensor("smallsB", [4, 4128], bf16, kind="ExternalInput")
    d["smallsC"] = nc.dram_tensor("smallsC", [4, 8256], bf16, kind="ExternalInput")
    d_out = nc.dram_tensor("out", [1, 32 * TL], f32, kind="ExternalOutput")

    XT_OFF = 0            # smallsA: xt4 [0 : 32T], wx_l1 [32T : 32T+4096]
    WXL1_OFF = 32 * T

    with tile.TileContext(nc) as tc:
        import contextlib

        with contextlib.ExitStack() as ctx:
            big = ctx.enter_context(tc.tile_pool(name="big", bufs=1))
            state = ctx.enter_context(tc.tile_pool(name="state", bufs=1))
            work = ctx.enter_context(tc.tile_pool(name="work", bufs=3))
            htp = ctx.enter_context(tc.tile_pool(name="htp", bufs=4))
            xpp = ctx.enter_context(tc.tile_pool(name="xpp", bufs=2))
            gp = ctx.enter_context(tc.tile_pool(name="gp", bufs=4, space="PSUM"))
            tp = ctx.enter_context(tc.tile_pool(name="tp", bufs=2, space="PSUM"))
            fbp = ctx.enter_context(tc.tile_pool(name="fbp", bufs=2, space="PSUM"))

            # ---- persistent tiles
            idw = big.tile([128, 144], bf16, tag="idw")
            smallsB = big.tile([4, 4128], bf16, tag="smB")
            smallsA = big.tile([4, 32 * T + 4096], bf16, tag="sA")
            wl1 = big.tile([128, 2, 4, 4, 512], bf16, tag="wB")
            hT1 = big.tile([128, 2, T, 128], bf16, tag="wD")
            nc.sync.dma_start(idw[:], d["idw"][:])
            nc.sync.dma_start(smallsB[:], d["smallsB"][:])
            nc.sync.dma_start(smallsA[:], d["smallsA"][:])
            for u in range(2):
                for r in range(4):
                    nc.sync.dma_start(wl1[:, u, r], d["wl1"][:, u, r])

            ident = idw[:, 0:128]
            ones4 = smallsB[0:4, 0:32]

            c_tiles = [state.tile([128, 128], f32, tag=f"c{u}", name=f"c{u}") for u in range(4)]

            def cell_ew(G, cu, s0, Hn_tag="Hn"):
                """Elementwise LSTM cell on G psum tile; returns Hn (bf16)."""
                S = work.tile([128, 384], bf16, tag="S")
                TG = work.tile([128, 128], bf16, tag="TG")
                nc.scalar.activation(S[:], G[:, 0:384], SIG)
                nc.scalar.activation(TG[:], G[:, 384:512], TANH)
                P1 = work.tile([128, 128], f32, tag="P1")
                nc.vector.tensor_mul(P1[:], S[:, 0:128], TG[:])
                if s0:
                    nc.vector.tensor_copy(cu[:], P1[:])
                else:
                    P2 = work.tile([128, 128], f32, tag="P2")
                    nc.vector.tensor_mul(P2[:], S[:, 128:256], cu[:])
                    nc.vector.tensor_add(cu[:], P1[:], P2[:])
                TC = work.tile([128, 128], bf16, tag="TC")
                nc.scalar.activation(TC[:], cu[:], TANH)
                Hn = work.tile([128, 128], bf16, tag=Hn_tag)
                nc.vector.tensor_mul(Hn[:], S[:, 256:384], TC[:])
                return Hn

            def transpose_h(Hn):
                HT = tp.tile([128, 128], bf16, tag="HT")
                nc.tensor.transpose(HT[:], Hn[:], ident)
                return HT

            MM = nc.tensor.matmul

            # ================= encoder layer 1 =================
            for s in range(T):
                for u in range(2):
                    t = s if u == 0 else T - 1 - s
                    t_prev = s - 1 if u == 0 else T - s
                    G = gp.tile([128, 512], f32, tag="G")
                    xt = smallsA[0:4, XT_OFF + 32 * t : XT_OFF + 32 * t + 32]
                    for q in range(4):
                        MM(G[32 * q : 32 * q + 32, :],
                           xt,
                           smallsA[0:4, WXL1_OFF + u * 2048 + 512 * q : WXL1_OFF + u * 2048 + 512 * (q + 1)],
                           start=True, stop=(s == 0),
                           tile_position=(0, 32 * q), skip_group_check=True)
                    if s > 0:
                        for r in range(4):
                            for q in range(4):
                                MM(G[32 * q : 32 * q + 32, :],
                                   hT1[:, u, t_prev, 32 * r : 32 * r + 32],
                                   wl1[:, u, r, q, :],
                                   start=False, stop=(r == 3),
                                   tile_position=(0, 32 * q), skip_group_check=True)
                    Hn = cell_ew(G, c_tiles[u], s == 0)
                    HT = transpose_h(Hn)
                    nc.vector.tensor_copy(hT1[:, u, t, :], HT[:])

            # ---- L2 weights (reuse slots of wl1 / new slot) ----
            wl2r = big.tile([128, 2, 4, 4, 512], bf16, tag="wB")
            wl2i = big.tile([128, 2, 8, 4, 512], bf16, tag="wC")
            for u in range(2):
                for r in range(4):
                    nc.sync.dma_start(wl2r[:, u, r], d["wl2r"][:, u, r])
                for r in range(8):
                    nc.sync.dma_start(wl2i[:, u, r], d["wl2i"][:, u, r])

            # ================= encoder layer 2 =================
            hT2_last = [None, None]
            for s in range(T):
                for u in range(2):
                    t = s if u == 0 else T - 1 - s
                    G = gp.tile([128, 512], f32, tag="G")
                    for k in range(8):
                        src_u, rr = (0, k) if k < 4 else (1, k - 4)
                        for q in range(4):
                            MM(G[32 * q : 32 * q + 32, :],
                               hT1[:, src_u, t, 32 * rr : 32 * rr + 32],
                               wl2i[:, u, k, q, :],
                               start=(k == 0), stop=False,
                               tile_position=(0, 32 * q), skip_group_check=True)
                    for q in range(4):
                        MM(G[32 * q : 32 * q + 32, :],
                           ones4,
                           smallsB[0:4, 32 + u * 2048 + 512 * q : 32 + u * 2048 + 512 * (q + 1)],
                           start=False, stop=(s == 0),
                           tile_position=(0, 32 * q), skip_group_check=True)
                    if s > 0:
                        hprev = hT2_last[u]
                        for r in range(4):
                            for q in range(4):
                                MM(G[32 * q : 32 * q + 32, :],
                                   hprev[:, 32 * r : 32 * r + 32],
                                   wl2r[:, u, r, q, :],
                                   start=False, stop=(r == 3),
                                   tile_position=(0, 32 * q), skip_group_check=True)
                    Hn = cell_ew(G, c_tiles[2 + u], s == 0)
                    HT = transpose_h(Hn)
                    h2 = htp.tile([128, 128], bf16, tag=f"h2_{u}")
                    nc.vector.tensor_copy(h2[:], HT[:])
                    hT2_last[u] = h2

            # ---- decoder init state (copy before slot reuse) ----
            hTd = [state.tile([128, 128], bf16, tag=f"hd{l}", name=f"hd{l}") for l in range(4)]
            nc.vector.tensor_copy(hTd[0][:], hT1[:, 0, T - 1, :])
            nc.vector.tensor_copy(hTd[1][:], hT1[:, 1, 0, :])
            nc.vector.tensor_copy(hTd[2][:], hT2_last[0][:])
            nc.vector.tensor_copy(hTd[3][:], hT2_last[1][:])
            c_dec = c_tiles  # layer l <- unit order [L1f, L1b, L2f, L2b]

            # ---- decoder weights (reuse slots) ----
            wdr = big.tile([128, 4, 4, 4, 512], bf16, tag="wD")
            wdi = big.tile([128, 3, 4, 4, 512], bf16, tag="wC")
            smallsC = big.tile([4, 8256], bf16, tag="sA")
            for l in range(4):
                for r in range(4):
                    nc.sync.dma_start(wdr[:, l, r], d["wdr"][:, l, r])
            for l in range(3):
                for r in range(4):
                    nc.sync.dma_start(wdi[:, l, r], d["wdi"][:, l, r])
            nc.sync.dma_start(smallsC[:], d["smallsC"][:])

            outbuf = state.tile([1, 32 * TL], f32, tag="outb")
            xp = xpp.tile([4, 32], bf16, tag="xp")
            nc.vector.tensor_copy(xp[:], smallsC[0:4, 8224:8256])

            # ================= decoder =================
            for td in range(TL):
                Gd = []
                # pre-rounds: Whh (+bias / +x) for all layers
                for l in range(4):
                    G = gp.tile([128, 512], f32, tag="G")
                    Gd.append(G)
                    for r in range(4):
                        for q in range(4):
                            MM(G[32 * q : 32 * q + 32, :],
                               hTd[l][:, 32 * r : 32 * r + 32],
                               wdr[:, l, r, q, :],
                               start=(r == 0), stop=False,
                               tile_position=(0, 32 * q), skip_group_check=True)
                    if l == 0:
                        for q in range(4):
                            MM(G[32 * q : 32 * q + 32, :],
                               xp,
                               smallsC[0:4, 512 * q : 512 * (q + 1)],
                               start=False, stop=True,
                               tile_position=(0, 32 * q), skip_group_check=True)
                    else:
                        for q in range(4):
                            MM(G[32 * q : 32 * q + 32, :],
                               ones4,
                               smallsC[0:4, 2048 + (l - 1) * 2048 + 512 * q : 2048 + (l - 1) * 2048 + 512 * (q + 1)],
                               start=False, stop=False,
                               tile_position=(0, 32 * q), skip_group_check=True)
                # chain
                for l in range(4):
                    if l > 0:
                        for r in range(4):
                            for q in range(4):
                                MM(Gd[l][32 * q : 32 * q + 32, :],
                                   hTd[l - 1][:, 32 * r : 32 * r + 32],
                                   wdi[:, l - 1, r, q, :],
                                   start=False, stop=(r == 3),
                                   tile_position=(0, 32 * q), skip_group_check=True)
                    Hn = cell_ew(Gd[l], c_dec[l], False)
                    HT = transpose_h(Hn)
                    hd_new = htp.tile([128, 128], bf16, tag=f"hd_cur{l}")
                    nc.vector.tensor_copy(hd_new[:], HT[:])
                    hTd[l] = hd_new
                # feedback: x_new^T [4, 32] = W* @ [h3; x_prev]
                Pfb = fbp.tile([4, 32], f32, tag="fb")
                for q in range(4):
                    MM(Pfb[:],
                       idw[:, 128 + 4 * q : 128 + 4 * (q + 1)],
                       hTd[3][:, 32 * q : 32 * q + 32],
                       start=(q == 0), stop=False,
                       tile_position=(0, 0), skip_group_check=True)
                MM(Pfb[:],
                   smallsC[0:4, 8192:8196],
                   xp,
                   start=False, stop=True,
                   tile_position=(0, 0), skip_group_check=True)
                xp = xpp.tile([4, 32], bf16, tag="xp")
                nc.vector.tensor_copy(xp[:], Pfb[:])
                nc.vector.tensor_copy(outbuf[0:1, 32 * td : 32 * td + 32], Pfb[0:1, :])

            nc.sync.dma_start(d_out[:], outbuf[:])

    nc.compile()
    return nc


# ---------------------------------------------------------------- entry
def _run(inputs, trace=False, tmpdir=None):
    enc_params = inputs["enc_params"]
    dec_params = inputs["dec_params"]
    lin_W = np.asarray(inputs["lin_W"], np.float32)
    lin_b = np.asarray(inputs["lin_b"], np.float32)
    x = np.asarray(inputs["x"], np.float32)
    tl = int(np.asarray(inputs["target_len"]))
    assert tl == TL, f"target_len {tl} != {TL}"
    assert x.shape == (FULL_B, T, IN_SZ), x.shape

    if "nc" not in _NC_CACHE:
        _NC_CACHE["nc"] = _build_nc()
    nc = _NC_CACHE["nc"]

    ws = _host_prep_weights(enc_params, dec_params, lin_W, lin_b)

    in_maps = []
    for s in range(N_CORES):
        bsl = slice(s * B, (s + 1) * B)
        xs = x[bsl]  # [32, T, 3]
        sa = np.zeros((4, 32 * T + 4096), np.float32)
        # xt4: [j, 32t+b]
        sa[0:3, 0 : 32 * T] = xs.transpose(2, 1, 0).reshape(3, T * 32)
        sa[3, 0 : 32 * T] = 1.0
        sa[:, 32 * T :] = ws["wx_l1_flat"]
        sc = ws["smallsC_base"].copy()
        sc[0:3, 8224:8256] = xs[:, T - 1, :].T
        sc[3, 8224:8256] = 1.0
        in_maps.append({
            "wl1": ws["wl1"], "wl2r": ws["wl2r"], "wl2i": ws["wl2i"],
            "wdr": ws["wdr"], "wdi": ws["wdi"], "idw": ws["idw"],
            "smallsA": sa.astype(BF16), "smallsB": ws["smallsB"],
            "smallsC": sc.astype(BF16),
        })

    from concourse.bass_utils import run_bass_kernel_spmd

    if trace:
        _install_ntff_hook()
    res = run_bass_kernel_spmd(
        nc, in_maps, core_ids=list(range(N_CORES)), trace=trace, tmpdir=tmpdir
    )
    out = np.empty((FULL_B, TL, 1), np.float32)
    for s in range(N_CORES):
        o = np.asarray(res.results[s]["out"]).reshape(TL, B)  # [t, b]
        out[s * B : (s + 1) * B, :, 0] = o.T
    return out, res


def _install_ntff_hook():
    import types
    import antenv

    if "antenv.axon_hooks" in sys.modules:
        return
    hooks_mod = types.ModuleType("antenv.axon_hooks")
    _HOOK = [None]
    hooks_mod.set_axon_ntff_profile_hook = lambda h: _HOOK.__setitem__(0, h)
    hooks_mod.get_axon_ntff_profile_hook = lambda: _HOOK[0]
    sys.modules["antenv.axon_hooks"] = hooks_mod
    antenv.axon_hooks = hooks_mod
    try:
        from trn_agent_boot.trn_boot import _ntff_profile_via_ctypes

        hooks_mod.set_axon_ntff_profile_hook(
            _ntff_profile_via_ctypes("/opt/axon/libaxon_pjrt.so")
        )
    except Exception:
        pass


def kernel(**inputs):
    out, _ = _run(inputs, trace=False)
    return out


# revision 6
# speedup vs baseline: 1.3804x; 1.3657x over previous
# Trainium2 Bass kernel for nn_BiLSTM_encoder_decoder (batch-sharded over 8 cores).
#
# Strategy (per core, batch shard B=32):
#   - All matmuls in bf16 with fp32 PSUM accumulation; cell state c kept fp32.
#   - "Quartered" gate layout: gates PSUM tile G[128, 512] where partition
#     rows = (quarter q, batch b) and free = (gate slot, within-quarter col).
#     Each of the 4 PE column-groups (tile_position=(0,32q)) computes one
#     quarter, so M=32 batch matmuls stream weights at full PE rate.
#   - h is kept transposed (hT[128, 128] = h.T arranged [hdim-in-quarter,
#     (quarter, batch)]) so it is directly the stationary operand of the next
#     step's matmul.  One full-mode 128x128 PE transpose per cell step
#     produces it.
#   - x inputs and biases folded in as an extra K=4 matmul round with a
#     constant-1 row.
#   - Decoder output linear + "decoder_sub" feedback folded into a single
#     5-round matmul producing the next decoder input [o0; s1; s2; 1]^T.
import os
import sys

for _p in ("/opt/trn_rl_repo", "/root/.axon_site/_ro/trn_rl_repo"):
    if os.path.isdir(_p) and _p not in sys.path:
        sys.path.append(_p)

import numpy as np
import ml_dtypes

BF16 = np.float16  # fp16: same PE rate/bytes as bf16, 8x finer mantissa

# ---- problem constants (hardcoded from the spec) ----
H = 512          # hidden
Q = 4            # quarters
B = 32           # batch per core
N_CORES = 8
FULL_B = 256
T = int(os.environ.get("BASS_LSTM_T", "128"))     # encoder seq len
TL = int(os.environ.get("BASS_LSTM_TL", "32"))    # decoder len
IN_SZ = 3

# gate slots in the free dim of G: slot0=i, slot1=f, slot2=o, slot3=g
ORIG_OF_SLOT = [0, 1, 3, 2]

_NC_CACHE = {}


# ---------------------------------------------------------------- host prep
def _prep_rounds(W):
    """W [2048, Kin] (pytorch gate order i,f,g,o) -> [R, 4q, 128, 512]."""
    Kin = W.shape[1]
    R = Kin // 128
    out = np.empty((R, 4, 128, 512), np.float32)
    for slot in range(4):
        orig = ORIG_OF_SLOT[slot]
        for q in range(4):
            blk = W[512 * orig + 128 * q : 512 * orig + 128 * (q + 1), :]  # [128cc, Kin]
            out[:, q, :, 128 * slot : 128 * (slot + 1)] = (
                blk.T.reshape(Kin // 128, 128, 128)
            )
    return out


def _prep_x(Wih, bias):
    """Wih [2048, n_in<=3] or None, bias [2048] -> [4 rows, 4q, 512].

    Row j<3 = Wih.T row j; row 3 = bias (multiplied by the constant-1 row)."""
    out = np.zeros((4, 4, 512), np.float32)
    for slot in range(4):
        orig = ORIG_OF_SLOT[slot]
        for q in range(4):
            sl = slice(512 * orig + 128 * q, 512 * orig + 128 * (q + 1))
            if Wih is not None:
                n_in = Wih.shape[1]
                out[0:n_in, q, 128 * slot : 128 * (slot + 1)] = Wih[sl, :].T
            out[3, q, 128 * slot : 128 * (slot + 1)] = bias[sl]
    return out


def _host_prep_weights(enc_params, dec_params, lin_W, lin_b):
    ws = {}
    # L1: 2 dirs. Whh rounds + x rounds.
    wl1 = np.empty((128, 2, 4, 4, 512), np.float32)
    wx_l1 = np.empty((4, 2, 4, 512), np.float32)
    for u in range(2):
        Wih, Whh, bih, bhh = [np.asarray(a, np.float32) for a in enc_params[0][u]]
        r = _prep_rounds(Whh)                      # [4, 4, 128, 512]
        wl1[:, u] = r.transpose(2, 0, 1, 3)        # [128, r, q, 512]
        wx_l1[:, u] = _prep_x(Wih, bih + bhh)
    # L2: rounds 0-3 Whh, 4-11 Wih(K=1024), bias-only x round.
    wl2r = np.empty((128, 2, 4, 4, 512), np.float32)
    wl2i = np.empty((128, 2, 8, 4, 512), np.float32)
    wx_l2b = np.empty((4, 2, 4, 512), np.float32)
    for u in range(2):
        Wih, Whh, bih, bhh = [np.asarray(a, np.float32) for a in enc_params[1][u]]
        wl2r[:, u] = _prep_rounds(Whh).transpose(2, 0, 1, 3)
        wl2i[:, u] = _prep_rounds(Wih).transpose(2, 0, 1, 3)
        wx_l2b[:, u] = _prep_x(None, bih + bhh)
    # decoder: 4 layers
    wdr = np.empty((128, 4, 4, 4, 512), np.float32)
    wdi = np.empty((128, 3, 4, 4, 512), np.float32)
    wx_db = np.empty((4, 3, 4, 512), np.float32)
    for l in range(4):
        Wih, Whh, bih, bhh = [np.asarray(a, np.float32) for a in dec_params[l]]
        wdr[:, l] = _prep_rounds(Whh).transpose(2, 0, 1, 3)
        if l == 0:
            wx_d0 = _prep_x(Wih, bih + bhh)        # [4, 4, 512]
        else:
            wdi[:, l - 1] = _prep_rounds(Wih).transpose(2, 0, 1, 3)
            wx_db[:, l - 1] = _prep_x(None, bih + bhh)
    # feedback W*: x_new^T [4, 32] = sum_q w_fb[:, q].T @ hT[:, q] + wx_fb.T @ x_prevT
    lw = np.asarray(lin_W, np.float32)[0]          # [512]
    lb = float(np.asarray(lin_b, np.float32)[0])
    w_fb = np.zeros((128, 16), np.float32)         # [kk, 4q + m]
    for q in range(4):
        seg = lw[128 * q : 128 * (q + 1)]
        w_fb[:, 4 * q + 0] = seg
        w_fb[:, 4 * q + 1] = -seg
        w_fb[:, 4 * q + 2] = seg
    wx_fb = np.zeros((4, 4), np.float32)
    wx_fb[0, 1] = 1.0
    wx_fb[0, 2] = -1.0
    wx_fb[1, 2] = 1.0
    wx_fb[3, 0] = lb
    wx_fb[3, 1] = -lb
    wx_fb[3, 2] = lb
    wx_fb[3, 3] = 1.0
    # identity for PE transpose + w_fb packed in one always-live tile
    idw = np.zeros((128, 144), np.float32)
    idw[:, 0:128] = np.eye(128, dtype=np.float32)
    idw[:, 128:144] = w_fb

    ws["wl1"] = wl1.astype(BF16)
    ws["wl2r"] = wl2r.astype(BF16)
    ws["wl2i"] = wl2i.astype(BF16)
    ws["wdr"] = wdr.astype(BF16)
    ws["wdi"] = wdi.astype(BF16)
    ws["idw"] = idw.astype(BF16)
    # smallsB [4, 4128]: ones432 [0:32] | wx_l2b [32:4128] (u*2048 + q*512)
    sb_ = np.zeros((4, 4128), np.float32)
    sb_[3, 0:32] = 1.0
    sb_[:, 32:4128] = wx_l2b.reshape(4, 4096)
    ws["smallsB"] = sb_.astype(BF16)
    # smallsC [4, 8256]: wx_d0 [0:2048] | wx_db [2048:8192] | wx_fb [8192:8196]
    # (x0t [8224:8256] is per-core, filled later)
    sc = np.zeros((4, 8256), np.float32)
    sc[:, 0:2048] = wx_d0.reshape(4, 2048)
    sc[:, 2048:8192] = wx_db.reshape(4, 6144)
    sc[:, 8192:8196] = wx_fb
    ws["smallsC_base"] = sc
    # smallsA [4, 4096 + 2048]: xt4 [0:32T] per-core | wx_l1 [32T : 32T+4096]
    ws["wx_l1_flat"] = wx_l1.reshape(4, 4096)
    return ws


# ---------------------------------------------------------------- device build
def _build_nc():
    import concourse.bass as bass
    import concourse.bacc as bacc
    import concourse.tile as tile
    from concourse import mybir

    f32 = mybir.dt.float32
    bf16 = mybir.dt.float16
    SIG = mybir.ActivationFunctionType.Sigmoid
    TANH = mybir.ActivationFunctionType.Tanh

    nc = bacc.Bacc("TRN2", target_bir_lowering=False, debug=False)

    d = {}
    d["wl1"] = nc.dram_tensor("wl1", [128, 2, 4, 4, 512], bf16, kind="ExternalInput")
    d["wl2r"] = nc.dram_tensor("wl2r", [128, 2, 4, 4, 512], bf16, kind="ExternalInput")
    d["wl2i"] = nc.dram_tensor("wl2i", [128, 2, 8, 4, 512], bf16, kind="ExternalInput")
    d["wdr"] = nc.dram_tensor("wdr", [128, 4, 4, 4, 512], bf16, kind="ExternalInput")
    d["wdi"] = nc.dram_tensor("wdi", [128, 3, 4, 4, 512], bf16, kind="ExternalInput")
    d["idw"] = nc.dram_tensor("idw", [128, 144], bf16, kind="ExternalInput")
    d["smallsA"] = nc.dram_tensor("smallsA", [4, 32 * T + 4096], bf16, kind="ExternalInput")
    d["smallsB"] = nc.dram_tensor("smallsB", [4, 4128], bf16, kind="ExternalInput")
    d["smallsC"] = nc.dram_tensor("smallsC", [4, 8256], bf16, kind="ExternalInput")
    d_out = nc.dram_tensor("out", [1, 32 * TL], f32, kind="ExternalOutput")

    XT_OFF = 0            # smallsA: xt4 [0 : 32T], wx_l1 [32T : 32T+4096]
    WXL1_OFF = 32 * T

    with tile.TileContext(nc) as tc:
        import contextlib

        with contextlib.ExitStack() as ctx:
            big = ctx.enter_context(tc.tile_pool(name="big", bufs=1))
            state = ctx.enter_context(tc.tile_pool(name="state", bufs=1))
            work = ctx.enter_context(tc.tile_pool(name="work", bufs=3))
            htp = ctx.enter_context(tc.tile_pool(name="htp", bufs=3))
            xpp = ctx.enter_context(tc.tile_pool(name="xpp", bufs=2))
            gp = ctx.enter_context(tc.tile_pool(name="gp", bufs=5, space="PSUM"))
            tp = ctx.enter_context(tc.tile_pool(name="tp", bufs=2, space="PSUM"))
            fbp = ctx.enter_context(tc.tile_pool(name="fbp", bufs=1, space="PSUM"))

            # ---- persistent tiles
            idw = big.tile([128, 144], bf16, tag="idw")
            smallsB = big.tile([4, 4128], bf16, tag="smB")
            smallsA = big.tile([4, 32 * T + 4096], bf16, tag="sA")
            wl1 = big.tile([128, 2, 4, 4, 512], bf16, tag="wB")
            hT1 = big.tile([128, 2, T, 128], bf16, tag="wD")
            nc.sync.dma_start(idw[:], d["idw"][:])
            nc.sync.dma_start(smallsB[:], d["smallsB"][:])
            nc.sync.dma_start(smallsA[:], d["smallsA"][:])
            for u in range(2):
                for r in range(4):
                    nc.sync.dma_start(wl1[:, u, r], d["wl1"][:, u, r])

            ident = idw[:, 0:128]
            ones4 = smallsB[0:4, 0:32]

            c_tiles = [state.tile([128, 128], f32, tag=f"c{u}", name=f"c{u}") for u in range(4)]

            def cell_ew(G, cu, s0, Hn_tag="Hn"):
                """Elementwise LSTM cell on G psum tile; returns Hn (bf16)."""
                S = work.tile([128, 384], bf16, tag="S")
                TG = work.tile([128, 128], bf16, tag="TG")
                nc.scalar.activation(S[:], G[:, 0:384], SIG)
                nc.scalar.activation(TG[:], G[:, 384:512], TANH)
                P1 = work.tile([128, 128], f32, tag="P1")
                nc.vector.tensor_mul(P1[:], S[:, 0:128], TG[:])
                if s0:
                    nc.vector.tensor_copy(cu[:], P1[:])
                else:
                    P2 = work.tile([128, 128], f32, tag="P2")
                    nc.vector.tensor_mul(P2[:], S[:, 128:256], cu[:])
                    nc.vector.tensor_add(cu[:], P1[:], P2[:])
                TC = work.tile([128, 128], bf16, tag="TC")
                nc.scalar.activation(TC[:], cu[:], TANH)
                Hn = work.tile([128, 128], bf16, tag=Hn_tag, bufs=2)
                nc.vector.tensor_mul(Hn[:], S[:, 256:384], TC[:])
                return Hn

            def transpose_h(Hn):
                HT = tp.tile([128, 128], bf16, tag="HT")
                nc.tensor.transpose(HT[:], Hn[:], ident)
                return HT

            MM = nc.tensor.matmul

            # ================= encoder layer 1 =================
            # Transpose+copy of step s-1 is emitted inside step s, after the
            # independent x-round, so the PE never stalls on the EW chain.
            pend1 = [None, None]
            for s in range(T):
                for u in range(2):
                    t = s if u == 0 else T - 1 - s
                    t_prev = s - 1 if u == 0 else T - s
                    G = gp.tile([128, 512], f32, tag="G")
                    xt = smallsA[0:4, XT_OFF + 32 * t : XT_OFF + 32 * t + 32]
                    for q in range(4):
                        MM(G[32 * q : 32 * q + 32, :],
                           xt,
                           smallsA[0:4, WXL1_OFF + u * 2048 + 512 * q : WXL1_OFF + u * 2048 + 512 * (q + 1)],
                           start=True, stop=(s == 0),
                           tile_position=(0, 32 * q), skip_group_check=True)
                    if s > 0:
                        Hn_prev = pend1[u]
                        HT = transpose_h(Hn_prev)
                        nc.vector.tensor_copy(hT1[:, u, t_prev, :], HT[:])
                        for r in range(4):
                            for q in range(4):
                                MM(G[32 * q : 32 * q + 32, :],
                                   hT1[:, u, t_prev, 32 * r : 32 * r + 32],
                                   wl1[:, u, r, q, :],
                                   start=False, stop=(r == 3),
                                   tile_position=(0, 32 * q), skip_group_check=True)
                    pend1[u] = cell_ew(G, c_tiles[u], s == 0, Hn_tag=f"Hn1_{u}")
            for u in range(2):
                t_last = T - 1 if u == 0 else 0
                HT = transpose_h(pend1[u])
                nc.vector.tensor_copy(hT1[:, u, t_last, :], HT[:])

            # ---- L2 weights (reuse slots of wl1 / new slot) ----
            wl2r = big.tile([128, 2, 4, 4, 512], bf16, tag="wB")
            wl2i = big.tile([128, 2, 8, 4, 512], bf16, tag="wC")
            for u in range(2):
                for r in range(4):
                    nc.sync.dma_start(wl2r[:, u, r], d["wl2r"][:, u, r])
                for r in range(8):
                    nc.sync.dma_start(wl2i[:, u, r], d["wl2i"][:, u, r])

            # ================= encoder layer 2 =================
            hT2_last = [None, None]
            pend2 = [None, None]
            for s in range(T):
                for u in range(2):
                    t = s if u == 0 else T - 1 - s
                    G = gp.tile([128, 512], f32, tag="G")
                    for k in range(8):
                        src_u, rr = (0, k) if k < 4 else (1, k - 4)
                        for q in range(4):
                            MM(G[32 * q : 32 * q + 32, :],
                               hT1[:, src_u, t, 32 * rr : 32 * rr + 32],
                               wl2i[:, u, k, q, :],
                               start=(k == 0), stop=False,
                               tile_position=(0, 32 * q), skip_group_check=True)
                    for q in range(4):
                        MM(G[32 * q : 32 * q + 32, :],
                           ones4,
                           smallsB[0:4, 32 + u * 2048 + 512 * q : 32 + u * 2048 + 512 * (q + 1)],
                           start=False, stop=(s == 0),
                           tile_position=(0, 32 * q), skip_group_check=True)
                    if s > 0:
                        Hn_prev = pend2[u]
                        HT = transpose_h(Hn_prev)
                        h2 = htp.tile([128, 128], bf16, tag=f"h2_{u}", name=f"h2_{u}")
                        nc.vector.tensor_copy(h2[:], HT[:])
                        hT2_last[u] = h2
                        for r in range(4):
                            for q in range(4):
                                MM(G[32 * q : 32 * q + 32, :],
                                   h2[:, 32 * r : 32 * r + 32],
                                   wl2r[:, u, r, q, :],
                                   start=False, stop=(r == 3),
                                   tile_position=(0, 32 * q), skip_group_check=True)
                    pend2[u] = cell_ew(G, c_tiles[2 + u], s == 0, Hn_tag=f"Hn2_{u}")
            for u in range(2):
                HT = transpose_h(pend2[u])
                h2 = htp.tile([128, 128], bf16, tag=f"h2_{u}", name=f"h2f_{u}")
                nc.vector.tensor_copy(h2[:], HT[:])
                hT2_last[u] = h2

            # ---- decoder init state (copy before slot reuse) ----
            hTd = [state.tile([128, 128], bf16, tag=f"hd{l}", name=f"hd{l}") for l in range(4)]
            nc.vector.tensor_copy(hTd[0][:], hT1[:, 0, T - 1, :])
            nc.vector.tensor_copy(hTd[1][:], hT1[:, 1, 0, :])
            nc.vector.tensor_copy(hTd[2][:], hT2_last[0][:])
            nc.vector.tensor_copy(hTd[3][:], hT2_last[1][:])
            c_dec = c_tiles  # layer l <- unit order [L1f, L1b, L2f, L2b]

            # ---- decoder weights (reuse slots) ----
            wdr = big.tile([128, 4, 4, 4, 512], bf16, tag="wD")
            wdi = big.tile([128, 3, 4, 4, 512], bf16, tag="wC")
            smallsC = big.tile([4, 8256], bf16, tag="sA")
            for l in range(4):
                for r in range(4):
                    nc.sync.dma_start(wdr[:, l, r], d["wdr"][:, l, r])
            for l in range(3):
                for r in range(4):
                    nc.sync.dma_start(wdi[:, l, r], d["wdi"][:, l, r])
            nc.sync.dma_start(smallsC[:], d["smallsC"][:])

            outbuf = state.tile([1, 32 * TL], f32, tag="outb")
            xp = xpp.tile([4, 32], bf16, tag="xp")
            nc.vector.tensor_copy(xp[:], smallsC[0:4, 8224:8256])

            # ================= decoder =================
            for td in range(TL):
                Gd = []
                # pre-rounds: Whh (+bias / +x) for all layers
                for l in range(4):
                    G = gp.tile([128, 512], f32, tag="G")
                    Gd.append(G)
                    for r in range(4):
                        for q in range(4):
                            MM(G[32 * q : 32 * q + 32, :],
                               hTd[l][:, 32 * r : 32 * r + 32],
                               wdr[:, l, r, q, :],
                               start=(r == 0), stop=False,
                               tile_position=(0, 32 * q), skip_group_check=True)
                    if l == 0:
                        for q in range(4):
                            MM(G[32 * q : 32 * q + 32, :],
                               xp,
                               smallsC[0:4, 512 * q : 512 * (q + 1)],
                               start=False, stop=True,
                               tile_position=(0, 32 * q), skip_group_check=True)
                    else:
                        for q in range(4):
                            MM(G[32 * q : 32 * q + 32, :],
                               ones4,
                               smallsC[0:4, 2048 + (l - 1) * 2048 + 512 * q : 2048 + (l - 1) * 2048 + 512 * (q + 1)],
                               start=False, stop=False,
                               tile_position=(0, 32 * q), skip_group_check=True)
                # chain
                for l in range(4):
                    if l > 0:
                        for r in range(4):
                            for q in range(4):
                                MM(Gd[l][32 * q : 32 * q + 32, :],
                                   hTd[l - 1][:, 32 * r : 32 * r + 32],
                                   wdi[:, l - 1, r, q, :],
                                   start=False, stop=(r == 3),
                                   tile_position=(0, 32 * q), skip_group_check=True)
                    Hn = cell_ew(Gd[l], c_dec[l], False)
                    HT = transpose_h(Hn)
                    hd_new = htp.tile([128, 128], bf16, tag=f"hd_cur{l}")
                    nc.vector.tensor_copy(hd_new[:], HT[:])
                    hTd[l] = hd_new
                # feedback: x_new^T [4, 32] = W* @ [h3; x_prev]
                Pfb = fbp.tile([4, 32], f32, tag="fb")
                for q in range(4):
                    MM(Pfb[:],
                       idw[:, 128 + 4 * q : 128 + 4 * (q + 1)],
                       hTd[3][:, 32 * q : 32 * q + 32],
                       start=(q == 0), stop=False,
                       tile_position=(0, 0), skip_group_check=True)
                MM(Pfb[:],
                   smallsC[0:4, 8192:8196],
                   xp,
                   start=False, stop=True,
                   tile_position=(0, 0), skip_group_check=True)
                xp = xpp.tile([4, 32], bf16, tag="xp")
                nc.vector.tensor_copy(xp[:], Pfb[:])
                nc.vector.tensor_copy(outbuf[0:1, 32 * td : 32 * td + 32], Pfb[0:1, :])

            nc.sync.dma_start(d_out[:], outbuf[:])

    nc.compile()
    return nc


# ---------------------------------------------------------------- entry
def _run(inputs, trace=False, tmpdir=None):
    enc_params = inputs["enc_params"]
    dec_params = inputs["dec_params"]
    lin_W = np.asarray(inputs["lin_W"], np.float32)
    lin_b = np.asarray(inputs["lin_b"], np.float32)
    x = np.asarray(inputs["x"], np.float32)
    tl = int(np.asarray(inputs["target_len"]))
    assert tl == TL, f"target_len {tl} != {TL}"
    assert x.shape == (FULL_B, T, IN_SZ), x.shape

    if "nc" not in _NC_CACHE:
        _NC_CACHE["nc"] = _build_nc()
    nc = _NC_CACHE["nc"]

    ws = _host_prep_weights(enc_params, dec_params, lin_W, lin_b)

    in_maps = []
    for s in range(N_CORES):
        bsl = slice(s * B, (s + 1) * B)
        xs = x[bsl]  # [32, T, 3]
        sa = np.zeros((4, 32 * T + 4096), np.float32)
        # xt4: [j, 32t+b]
        sa[0:3, 0 : 32 * T] = xs.transpose(2, 1, 0).reshape(3, T * 32)
        sa[3, 0 : 32 * T] = 1.0
        sa[:, 32 * T :] = ws["wx_l1_flat"]
        sc = ws["smallsC_base"].copy()
        sc[0:3, 8224:8256] = xs[:, T - 1, :].T
        sc[3, 8224:8256] = 1.0
        in_maps.append({
            "wl1": ws["wl1"], "wl2r": ws["wl2r"], "wl2i": ws["wl2i"],
            "wdr": ws["wdr"], "wdi": ws["wdi"], "idw": ws["idw"],
            "smallsA": sa.astype(BF16), "smallsB": ws["smallsB"],
            "smallsC": sc.astype(BF16),
        })

    from concourse.bass_utils import run_bass_kernel_spmd

    if trace:
        _install_ntff_hook()
    res = run_bass_kernel_spmd(
        nc, in_maps, core_ids=list(range(N_CORES)), trace=trace, tmpdir=tmpdir
    )
    out = np.empty((FULL_B, TL, 1), np.float32)
    for s in range(N_CORES):
        o = np.asarray(res.results[s]["out"]).reshape(TL, B)  # [t, b]
        out[s * B : (s + 1) * B, :, 0] = o.T
    return out, res


def _install_ntff_hook():
    import types
    import antenv

    if "antenv.axon_hooks" in sys.modules:
        return
    hooks_mod = types.ModuleType("antenv.axon_hooks")
    _HOOK = [None]
    hooks_mod.set_axon_ntff_profile_hook = lambda h: _HOOK.__setitem__(0, h)
    hooks_mod.get_axon_ntff_profile_hook = lambda: _HOOK[0]
    sys.modules["antenv.axon_hooks"] = hooks_mod
    antenv.axon_hooks = hooks_mod
    try:
        from trn_agent_boot.trn_boot import _ntff_profile_via_ctypes

        hooks_mod.set_axon_ntff_profile_hook(
            _ntff_profile_via_ctypes("/opt/axon/libaxon_pjrt.so")
        )
    except Exception:
        pass


def kernel(**inputs):
    out, _ = _run(inputs, trace=False)
    return out
